# revision 22
# baseline (speedup 1.0000x reference)
"""Trainium2 Bass kernel for the FF-NeRF MLP (nn_FFNerfModel_3092376453816).

Data-parallel over 8 NeuronCores: points sharded along N, weights replicated.
Feature-major on-chip layout: activations are [d, n_tile] with the feature dim
on SBUF partitions, so every layer is a single TensorE matmul with the weight
as the stationary operand and no transposes anywhere.

Positional encoding: the host pre-computes exactly-wrapped sin arguments
(2^j * x mod 2pi, in float64 -- exact because fp32(2^j*x) is exact), so the
whole embedding is ONE ScalarE Sin op per 2048-point iteration (the HW sin
table only covers [-pi, pi]).  Raw coords ride along as sin(eps*x)/eps with
the 1/eps folded into the first-layer weights.

Per 2048-point iteration, FOUR independent 512-point chains are interleaved
(each with its own 2-slot PSUM pool; 8 banks total) so the serial
matmul->PSUM-evacuation dependency chain of one chain overlaps the others;
PSUM evacuation (relu+fp16 cast) is split between VectorE and ScalarE.  All
matmuls are fp16 (fp32 is 4x slower on the PE); PSUM accumulates fp32.  The
b2_w3 @ b3_w0[:128] linear pair is fused host-side, and sigma rides as
column 65 of that fused matmul, so block2's output layer and block3's input
layer cost one matmul stream.  sigmoid(x) = 0.5 + 0.5*tanh(x/2) keeps the
whole kernel on one activation-table set (sin+tanh+relu in silu_and_others).

Measured: ~1.12 ms NEFF exec on 8 cores, color l2 rel-err 2.2e-4,
sigma 7.9e-4 vs the fp32 reference.
"""

import os
import subprocess
import sys
import tempfile

import numpy as np

N = 1048576
NCORES = 8
NCORE = N // NCORES          # 131072 points per core
NT = 512                     # points per tile (one PSUM bank of fp32)
NTILES = NCORE // NT         # 256
L_POS, L_DIR, H = 10, 4, 128
D_POS = 3 + 2 * 3 * L_POS    # 63
D_DIR = 3 + 2 * 3 * L_DIR    # 27
EPS = 2.0 ** -8              # raw-coord sin trick: sin(eps*x) ~= eps*x
TWO_PI = 2.0 * np.pi

# Rows of the on-chip embedding tile E [90, n]:
#   0..2    eps*pos               (raw coords via sin-linearization)
#   3..32   sin(2^j pos_c)        r = 3 + 3j + c
#   33..62  cos(2^j pos_c)        r = 33 + 3j + c
#   63      zero pad (AP base-partition must be 0/32/64)
#   64..66  eps*dirs
#   67..78  sin(2^j dirs_c)
#   79..90  cos(2^j dirs_c)
# x_emb = rows 0..62 (contiguous), d_emb = rows 64..90 (contiguous).


def _perm(L):
    """my-local-row -> reference-row for one posenc block of size 3+6L."""
    d = 3 + 6 * L
    p = np.zeros(d, dtype=np.int64)
    p[0:3] = [0, 1, 2]
    for j in range(L):
        for c in range(3):
            p[3 + 3 * j + c] = 3 + 6 * j + c          # sin rows
            p[3 + 3 * L + 3 * j + c] = 6 + 6 * j + c  # cos rows
    return p


def _permute_w(w_ref, L):
    """Permute posenc-facing weight rows to my layout; scale raw rows by 1/eps."""
    w = np.asarray(w_ref, dtype=np.float64)[_perm(L)]
    w[0:3] *= 1.0 / EPS
    return w


def _build_r(pos, dirs):
    """R [90, N] float32: pre-wrapped sin arguments for the big Sin op."""
    r = np.empty((91, pos.shape[0]), dtype=np.float32)
    r[63] = 0.0
    pos_t = np.asarray(pos, dtype=np.float64).T     # [3, N]
    dir_t = np.asarray(dirs, dtype=np.float64).T

    def wrap(t):
        return (t - TWO_PI * np.round(t / TWO_PI)).astype(np.float32)

    r[0:3] = (EPS * pos_t).astype(np.float32)
    r[64:67] = (EPS * dir_t).astype(np.float32)
    for j in range(L_POS):
        arg = (2.0 ** j) * pos_t
        r[3 + 3 * j: 6 + 3 * j] = wrap(arg)
        r[33 + 3 * j: 36 + 3 * j] = wrap(arg + np.pi / 2)
    for j in range(L_DIR):
        arg = (2.0 ** j) * dir_t
        r[67 + 3 * j: 70 + 3 * j] = wrap(arg)
        r[79 + 3 * j: 82 + 3 * j] = wrap(arg + np.pi / 2)
    return r


# Column offsets of each stationary weight inside the packed [128, 1285] tile.
W_OFF = {
    "w0": (0, 63, 128, 0),      # (col offset, K, M, base partition)
    "w1": (128, 128, 128, 0),
    "w2": (256, 128, 128, 0),
    "w3": (384, 128, 128, 0),
    "w4": (512, 128, 128, 0),
    "w5a": (640, 128, 128, 0),
    "w5b": (768, 63, 128, 0),
    "w6": (896, 128, 128, 0),
    "w7": (1024, 128, 128, 0),
    "w8": (1152, 128, 65, 0),
    "w9": (1217, 27, 65, 64),   # moving operand d_emb starts at partition 64
    "w10": (1282, 64, 3, 0),
}
W_COLS = 1285


def _pack_weights(ws):
    wpack = np.zeros((128, W_COLS), dtype=np.float16)
    for name, mat in ws.items():
        off, k, m, base = W_OFF[name]
        assert mat.shape == (k, m), (name, mat.shape)
        wpack[base:base + k, off:off + m] = mat.astype(np.float16)
    return wpack


def _prep_weights(b1_w0, b1_w1, b1_w2, b1_w3, b1_w4,
                  b2_w0, b2_w1, b2_w2, b2_w3, b3_w0, b3_w1):
    f64 = lambda a: np.asarray(a, dtype=np.float64)
    # Fuse the two linear layers around `feat`: feat = g2 @ b2_w3[:, :128]
    # (no activation) feeds b3_w0[:128] directly.
    w_fused = f64(b2_w3)[:, :128] @ f64(b3_w0)[:128, :]      # [128, 64]
    w8 = np.concatenate([w_fused, f64(b2_w3)[:, 128:129]], axis=1)  # [128, 65]
    w9 = np.zeros((27, 65), dtype=np.float64)
    w9[:, 0:64] = _permute_w(f64(b3_w0)[128:], L_DIR)        # d_emb -> c
    ws = {
        "w0": _permute_w(b1_w0, L_POS),
        "w1": f64(b1_w1), "w2": f64(b1_w2), "w3": f64(b1_w3), "w4": f64(b1_w4),
        "w5a": f64(b2_w0)[:128], "w5b": _permute_w(f64(b2_w0)[128:], L_POS),
        "w6": f64(b2_w1), "w7": f64(b2_w2),
        "w8": w8, "w9": w9, "w10": f64(b3_w1),
    }
    return _pack_weights(ws)


# ---------------------------------------------------------------------------
# Device side (runs in a subprocess so the host process's jax state/platform
# selection can't interfere with the axon PJRT backend).
# ---------------------------------------------------------------------------

def _install_trace_shims():
    """The image's antenv lacks axon_hooks; replicate trn_boot's ctypes hook
    so run_bass_kernel_spmd(trace=True) can capture NTFF profiles."""
    import contextlib
    import ctypes
    import types

    lib = ctypes.CDLL("/opt/axon/libaxon_pjrt.so")
    if not hasattr(lib, "axon_start_nrt_profile"):
        return
    lib.axon_start_nrt_profile.argtypes = [ctypes.POINTER(ctypes.c_int64),
                                           ctypes.c_size_t]
    lib.axon_start_nrt_profile.restype = ctypes.c_int64
    lib.axon_stop_nrt_profile.argtypes = [ctypes.c_char_p]
    lib.axon_stop_nrt_profile.restype = ctypes.c_int64

    @contextlib.contextmanager
    def _hook(output_dir, device_ids):
        import jax
        jax.devices()
        if device_ids:
            ids = (ctypes.c_int64 * len(device_ids))(*device_ids)
            rc = lib.axon_start_nrt_profile(ids, len(device_ids))
        else:
            rc = lib.axon_start_nrt_profile(None, 0)
        if rc != 0:
            raise RuntimeError(f"axon_start_nrt_profile rc={rc}")
        try:
            yield
        finally:
            n = lib.axon_stop_nrt_profile(str(output_dir).encode())
            print(f"ntff profile: {n} file(s) -> {output_dir}", file=sys.stderr)

    mod = types.ModuleType("antenv.axon_hooks")
    mod.get_axon_ntff_profile_hook = lambda: _hook
    mod.set_axon_ntff_profile_hook = lambda h: None
    import antenv
    sys.modules["antenv.axon_hooks"] = mod
    antenv.axon_hooks = mod
    import concourse.bass_utils as bu
    bu.upload_artifacts = lambda tmpdir: tmpdir


def _build_and_run(r_full, wpack, trace):
    sys.path.insert(0, "/opt/trn_rl_repo")
    import concourse.bass as bass
    import concourse.tile as tile
    from concourse import bacc, mybir
    from concourse.bass_utils import run_bass_kernel_spmd

    f32, f16 = mybir.dt.float32, mybir.dt.float16
    AF = mybir.ActivationFunctionType
    ALU = mybir.AluOpType

    # Pin all activations (Sin/Tanh/Relu) to the one table set that holds
    # them all -- otherwise the table-load placement pass alternates sets
    # between Sin and Tanh every tile (~2.7us per switch).
    from concourse.hw_specs import get_activation_tables
    for _name, _funcs in get_activation_tables("gen3").items():
        if _name not in ("silu_and_others",):
            _funcs.discard(AF.Sin)
            _funcs.discard(AF.Tanh)

    NCH = 4                       # interleaved point-chains per iteration
    IT = NCH * NT                 # 2048 points per iteration
    NITER = NCORE // IT           # 64

    # Evacuation engine per (chain, stage): DVE gets 21 of 36, ACT 15
    # (ACT also runs the big Sin and the tanh outputs).
    EVAC_ENG = {}
    for c in range(NCH):
        for L in range(9):
            # Alternate engines along each chain (consecutive stage evacs hit
            # different queues), staggered across chains; 20 DVE / 16 ACT.
            EVAC_ENG[(c, L)] = "v" if (L + c) % 2 == 0 else "s"
    EVAC_ENG[(1, 8)] = "v"
    EVAC_ENG[(3, 8)] = "v"

    nc = bacc.Bacc("TRN2", target_bir_lowering=False, debug=False,
                   num_devices=NCORES)
    r_dram = nc.declare_dram_parameter("r", [91, NCORE], f32, isOutput=False)
    w_dram = nc.declare_dram_parameter("w", [128, W_COLS], f16, isOutput=False)
    oc_dram = nc.declare_dram_parameter("oc", [3, NCORE], f16, isOutput=True)
    os_dram = nc.declare_dram_parameter("os", [1, NCORE], f16, isOutput=True)

    def w_ap(wtile, name):
        off, k, m, base = W_OFF[name]
        return wtile[base:base + k, off:off + m]

    with tile.TileContext(nc) as tc:
        with (
            tc.tile_pool(name="wpool", bufs=1) as wpool,
            tc.tile_pool(name="rpool", bufs=4) as rpool,
            tc.tile_pool(name="epool", bufs=4) as epool,
            tc.tile_pool(name="hpool", bufs=6) as hpool,
            tc.tile_pool(name="cpool", bufs=4) as cpool,
            tc.tile_pool(name="opool", bufs=4) as opool,
            tc.tile_pool(name="p0", bufs=2, space=bass.MemorySpace.PSUM) as pp0,
            tc.tile_pool(name="p1", bufs=2, space=bass.MemorySpace.PSUM) as pp1,
            tc.tile_pool(name="p2", bufs=2, space=bass.MemorySpace.PSUM) as pp2,
            tc.tile_pool(name="p3", bufs=2, space=bass.MemorySpace.PSUM) as pp3,
        ):
            ppools = [pp0, pp1, pp2, pp3]
            wtile = wpool.tile([128, W_COLS], f16)
            nc.sync.dma_start(wtile[:], w_dram[:])

            prev = None   # deferred color head input
            def color_head(prev):
                c65p, Sp, itp = prev
                ot3 = opool.tile([35, IT // 2], f16, tag="ot3")
                t3a = opool.tile([35, IT // 2], f32, tag="t3a")
                # Stack pairs of chains at partition bases 0 and 32 of one
                # PSUM tile so two tanh ops cover all four chains.
                for pr in range(2):
                    hc = bass.ts(pr, IT // 4)   # cols for this pair in t3a
                    po = ppools[pr].tile([35, NT], f32, tag=f"p{pr}")
                    for half in range(2):
                        c = 2 * pr + half
                        cs = bass.ts(c, NT)
                        base = 32 * half
                        nc.tensor.matmul(po[base:base + 3, :],
                                         w_ap(wtile, "w10"), c65p[0:64, cs],
                                         start=True, stop=True,
                                         skip_group_check=True)
                    nc.scalar.activation(t3a[:, hc], po[:], AF.Tanh, scale=0.5)
                nc.gpsimd.tensor_scalar(ot3[:], t3a[:], 0.5, 0.5,
                                        ALU.mult, ALU.add)
                for pr in range(2):
                    for half in range(2):
                        c = 2 * pr + half
                        base = 32 * half
                        col = pr * NT
                        nc.sync.dma_start(
                            oc_dram[:, bass.ts(itp * NCH + c, NT)],
                            ot3[base:base + 3, col:col + NT])
                nc.sync.dma_start(os_dram[:, Sp], c65p[64:65, :])

            for it in range(NITER):
                S = bass.ts(it, IT)
                rt = rpool.tile([91, IT], f32, tag="rt")
                nc.sync.dma_start(rt[:], r_dram[:, S])
                et = epool.tile([91, IT], f16, tag="et")
                nc.scalar.activation(et[:], rt[:], AF.Sin)

                if prev is not None:
                    color_head(prev)

                x_emb = [et[0:63, bass.ts(c, NT)] for c in range(NCH)]
                d_emb = [et[64:91, bass.ts(c, NT)] for c in range(NCH)]

                h = None      # shared [128, IT] tile, one per stage
                c65 = None    # shared [65, IT]

                def stage_mm(c, L):
                    pool = ppools[c]
                    cs = bass.ts(c, NT)
                    if L == 0:
                        p = pool.tile([128, NT], f32, tag=f"p{c}")
                        nc.tensor.matmul(p[:], w_ap(wtile, "w0"), x_emb[c],
                                         start=True, stop=True)
                    elif L in (1, 2, 3, 4, 6, 7):
                        p = pool.tile([128, NT], f32, tag=f"p{c}")
                        nc.tensor.matmul(p[:], w_ap(wtile, f"w{L}"),
                                         h[:, cs], start=True, stop=True)
                    elif L == 5:
                        p = pool.tile([128, NT], f32, tag=f"p{c}")
                        nc.tensor.matmul(p[:], w_ap(wtile, "w5a"), h[:, cs],
                                         start=True, stop=False)
                        nc.tensor.matmul(p[:], w_ap(wtile, "w5b"), x_emb[c],
                                         start=False, stop=True)
                    else:  # L == 8: c_pre [65, NT] (+ sigma row)
                        p = pool.tile([65, NT], f32, tag=f"p{c}")
                        nc.tensor.matmul(p[:], w_ap(wtile, "w8"), h[:, cs],
                                         start=True, stop=False)
                        nc.tensor.matmul(p[:], w_ap(wtile, "w9"), d_emb[c],
                                         start=False, stop=True)
                    return p

                for L in range(9):
                    ps = [stage_mm(c, L) for c in range(NCH)]
                    if L == 8:
                        dst = cpool.tile([65, IT], f16, tag="c65")
                    else:
                        dst = hpool.tile([128, IT], f16, tag="h")
                    for c in range(NCH):
                        cs = bass.ts(c, NT)
                        if EVAC_ENG[(c, L)] == "v":
                            nc.vector.tensor_scalar_max(dst[:, cs], ps[c][:], 0.0)
                        else:
                            nc.scalar.activation(dst[:, cs], ps[c][:], AF.Relu)
                    if L == 8:
                        c65 = dst
                    else:
                        h = dst

                prev = (c65, S, it)

            color_head(prev)

    nc.compile()

    in_maps = []
    for c in range(NCORES):
        rc = np.ascontiguousarray(r_full[:, c * NCORE:(c + 1) * NCORE])
        in_maps.append({"r": rc, "w": wpack})
    kw = {}
    if trace:
        _install_trace_shims()
        trace_dir = os.environ.get("FFNERF_TRACE_DIR", "/tmp/ffnerf_trace")
        os.makedirs(trace_dir, exist_ok=True)
        kw["tmpdir"] = trace_dir
    res = run_bass_kernel_spmd(nc, in_maps, list(range(NCORES)), trace=trace, **kw)
    if trace and res.instructions_and_trace is not None:
        print("perfetto trace:", res.instructions_and_trace[1], file=sys.stderr)
    oc = np.concatenate([res.results[c]["oc"] for c in range(NCORES)], axis=1)
    osig = np.concatenate([res.results[c]["os"] for c in range(NCORES)], axis=1)
    return oc, osig, res


def _run_device_subprocess(r_full, wpack, trace):
    tmpdir = tempfile.mkdtemp(prefix="ffnerf_")
    in_path = os.path.join(tmpdir, "in.npz")
    out_path = os.path.join(tmpdir, "out.npz")
    np.savez(in_path, r=r_full, w=wpack)
    env = dict(os.environ)
    env.pop("JAX_PLATFORMS", None)
    env["FFNERF_TRACE"] = "1" if trace else "0"
    cmd = [sys.executable, os.path.abspath(__file__), "--device-run",
           in_path, out_path]
    # Device sessions occasionally fail transiently (e.g. a prior session
    # still releasing the cores) -- retry a few times.
    import time
    last = None
    for attempt in range(4):
        try:
            subprocess.run(cmd, check=True, env=env)
            break
        except subprocess.CalledProcessError as e:
            last = e
            time.sleep(10 * (attempt + 1))
    else:
        raise last
    d = np.load(out_path)
    return d["oc"], d["os"], d["exec_ns"]


def kernel(pos, dirs, b1_w0, b1_w1, b1_w2, b1_w3, b1_w4,
           b2_w0, b2_w1, b2_w2, b2_w3, b3_w0, b3_w1):
    pos = np.asarray(pos, dtype=np.float32)
    dirs = np.asarray(dirs, dtype=np.float32)
    assert pos.shape == (N, 3) and dirs.shape == (N, 3)
    wpack = _prep_weights(b1_w0, b1_w1, b1_w2, b1_w3, b1_w4,
                          b2_w0, b2_w1, b2_w2, b2_w3, b3_w0, b3_w1)
    r_full = _build_r(pos, dirs)
    trace = os.environ.get("FFNERF_TRACE", "0") == "1"
    oc, osig, exec_ns = _run_device_subprocess(r_full, wpack, trace)
    global LAST_EXEC_NS
    LAST_EXEC_NS = exec_ns
    color = np.ascontiguousarray(oc.T).astype(np.float32)        # [N, 3]
    sigma = osig[0].astype(np.float32)                           # [N]
    return color, sigma


LAST_EXEC_NS = None


if __name__ == "__main__" and len(sys.argv) >= 4 and sys.argv[1] == "--device-run":
    d = np.load(sys.argv[2])
    trace = os.environ.get("FFNERF_TRACE", "0") == "1"
    oc, osig, res = _build_and_run(d["r"], d["w"], trace)
    exec_ns = res.exec_time_ns if res.exec_time_ns is not None else -1
    np.savez(sys.argv[3], oc=oc, os=osig, exec_ns=np.int64(exec_ns))
    print(f"device-run done, exec_time_ns={exec_ns}")


# revision 23
# speedup vs baseline: 1.1201x; 1.1201x over previous
"""Trainium2 Bass kernel for the FF-NeRF MLP (nn_FFNerfModel_3092376453816).

Data-parallel over 8 NeuronCores: points sharded along N, weights replicated.
Feature-major on-chip layout: activations are [d, n_tile] with the feature dim
on SBUF partitions, so every layer is a single TensorE matmul with the weight
as the stationary operand and no transposes anywhere.

Positional encoding: the host pre-computes exactly-wrapped sin arguments
(2^j * x mod 2pi, in float64 -- exact because fp32(2^j*x) is exact), so the
whole embedding is ONE ScalarE Sin op per 2048-point iteration (the HW sin
table only covers [-pi, pi]).  Raw coords ride along as sin(eps*x)/eps with
the 1/eps folded into the first-layer weights.

Per 2048-point iteration, FOUR independent 512-point chains are interleaved
(each with its own 2-slot PSUM pool; 8 banks total) so the serial
matmul->PSUM-evacuation dependency chain of one chain overlaps the others;
PSUM evacuation (relu+fp16 cast) is split between VectorE and ScalarE.  All
matmuls are fp16 (fp32 is 4x slower on the PE); PSUM accumulates fp32.  The
b2_w3 @ b3_w0[:128] linear pair is fused host-side, and sigma rides as
column 65 of that fused matmul, so block2's output layer and block3's input
layer cost one matmul stream.  sigmoid(x) = 0.5 + 0.5*tanh(x/2) keeps the
whole kernel on one activation-table set (sin+tanh+relu in silu_and_others).

Measured: ~1.12 ms NEFF exec on 8 cores, color l2 rel-err 2.2e-4,
sigma 7.9e-4 vs the fp32 reference.
"""

import os
import subprocess
import sys
import tempfile

import numpy as np

N = 1048576
NCORES = 8
NCORE = N // NCORES          # 131072 points per core
NT = 512                     # points per tile (one PSUM bank of fp32)
NTILES = NCORE // NT         # 256
L_POS, L_DIR, H = 10, 4, 128
D_POS = 3 + 2 * 3 * L_POS    # 63
D_DIR = 3 + 2 * 3 * L_DIR    # 27
EPS = 2.0 ** -8              # raw-coord sin trick: sin(eps*x) ~= eps*x
TWO_PI = 2.0 * np.pi

# Rows of the on-chip embedding tile E [90, n]:
#   0..2    eps*pos               (raw coords via sin-linearization)
#   3..32   sin(2^j pos_c)        r = 3 + 3j + c
#   33..62  cos(2^j pos_c)        r = 33 + 3j + c
#   63      zero pad (AP base-partition must be 0/32/64)
#   64..66  eps*dirs
#   67..78  sin(2^j dirs_c)
#   79..90  cos(2^j dirs_c)
# x_emb = rows 0..62 (contiguous), d_emb = rows 64..90 (contiguous).


def _perm(L):
    """my-local-row -> reference-row for one posenc block of size 3+6L."""
    d = 3 + 6 * L
    p = np.zeros(d, dtype=np.int64)
    p[0:3] = [0, 1, 2]
    for j in range(L):
        for c in range(3):
            p[3 + 3 * j + c] = 3 + 6 * j + c          # sin rows
            p[3 + 3 * L + 3 * j + c] = 6 + 6 * j + c  # cos rows
    return p


def _permute_w(w_ref, L):
    """Permute posenc-facing weight rows to my layout; scale raw rows by 1/eps."""
    w = np.asarray(w_ref, dtype=np.float64)[_perm(L)]
    w[0:3] *= 1.0 / EPS
    return w


def _build_r(pos, dirs):
    """R [90, N] float32: pre-wrapped sin arguments for the big Sin op."""
    r = np.empty((91, pos.shape[0]), dtype=np.float32)
    r[63] = 0.0
    pos_t = np.asarray(pos, dtype=np.float64).T     # [3, N]
    dir_t = np.asarray(dirs, dtype=np.float64).T

    def wrap(t):
        return (t - TWO_PI * np.round(t / TWO_PI)).astype(np.float32)

    r[0:3] = (EPS * pos_t).astype(np.float32)
    r[64:67] = (EPS * dir_t).astype(np.float32)
    for j in range(L_POS):
        arg = (2.0 ** j) * pos_t
        r[3 + 3 * j: 6 + 3 * j] = wrap(arg)
        r[33 + 3 * j: 36 + 3 * j] = wrap(arg + np.pi / 2)
    for j in range(L_DIR):
        arg = (2.0 ** j) * dir_t
        r[67 + 3 * j: 70 + 3 * j] = wrap(arg)
        r[79 + 3 * j: 82 + 3 * j] = wrap(arg + np.pi / 2)
    return r


# Column offsets of each stationary weight inside the packed [128, 1285] tile.
W_OFF = {
    "w0": (0, 63, 128, 0),      # (col offset, K, M, base partition)
    "w1": (128, 128, 128, 0),
    "w2": (256, 128, 128, 0),
    "w3": (384, 128, 128, 0),
    "w4": (512, 128, 128, 0),
    "w5a": (640, 128, 128, 0),
    "w5b": (768, 63, 128, 0),
    "w6": (896, 128, 128, 0),
    "w7": (1024, 128, 128, 0),
    "w8": (1152, 128, 65, 0),
    "w9": (1217, 27, 65, 64),   # moving operand d_emb starts at partition 64
    "w10": (1282, 64, 3, 0),
}
W_COLS = 1285


def _pack_weights(ws):
    wpack = np.zeros((128, W_COLS), dtype=np.float16)
    for name, mat in ws.items():
        off, k, m, base = W_OFF[name]
        assert mat.shape == (k, m), (name, mat.shape)
        wpack[base:base + k, off:off + m] = mat.astype(np.float16)
    return wpack


def _prep_weights(b1_w0, b1_w1, b1_w2, b1_w3, b1_w4,
                  b2_w0, b2_w1, b2_w2, b2_w3, b3_w0, b3_w1):
    f64 = lambda a: np.asarray(a, dtype=np.float64)
    # Fuse the two linear layers around `feat`: feat = g2 @ b2_w3[:, :128]
    # (no activation) feeds b3_w0[:128] directly.
    w_fused = f64(b2_w3)[:, :128] @ f64(b3_w0)[:128, :]      # [128, 64]
    w8 = np.concatenate([w_fused, f64(b2_w3)[:, 128:129]], axis=1)  # [128, 65]
    w9 = np.zeros((27, 65), dtype=np.float64)
    w9[:, 0:64] = _permute_w(f64(b3_w0)[128:], L_DIR)        # d_emb -> c
    ws = {
        "w0": _permute_w(b1_w0, L_POS),
        "w1": f64(b1_w1), "w2": f64(b1_w2), "w3": f64(b1_w3), "w4": f64(b1_w4),
        "w5a": f64(b2_w0)[:128], "w5b": _permute_w(f64(b2_w0)[128:], L_POS),
        "w6": f64(b2_w1), "w7": f64(b2_w2),
        "w8": w8, "w9": w9, "w10": f64(b3_w1),
    }
    return _pack_weights(ws)


# ---------------------------------------------------------------------------
# Device side (runs in a subprocess so the host process's jax state/platform
# selection can't interfere with the axon PJRT backend).
# ---------------------------------------------------------------------------

def _install_trace_shims():
    """The image's antenv lacks axon_hooks; replicate trn_boot's ctypes hook
    so run_bass_kernel_spmd(trace=True) can capture NTFF profiles."""
    import contextlib
    import ctypes
    import types

    lib = ctypes.CDLL("/opt/axon/libaxon_pjrt.so")
    if not hasattr(lib, "axon_start_nrt_profile"):
        return
    lib.axon_start_nrt_profile.argtypes = [ctypes.POINTER(ctypes.c_int64),
                                           ctypes.c_size_t]
    lib.axon_start_nrt_profile.restype = ctypes.c_int64
    lib.axon_stop_nrt_profile.argtypes = [ctypes.c_char_p]
    lib.axon_stop_nrt_profile.restype = ctypes.c_int64

    @contextlib.contextmanager
    def _hook(output_dir, device_ids):
        import jax
        jax.devices()
        if device_ids:
            ids = (ctypes.c_int64 * len(device_ids))(*device_ids)
            rc = lib.axon_start_nrt_profile(ids, len(device_ids))
        else:
            rc = lib.axon_start_nrt_profile(None, 0)
        if rc != 0:
            raise RuntimeError(f"axon_start_nrt_profile rc={rc}")
        try:
            yield
        finally:
            n = lib.axon_stop_nrt_profile(str(output_dir).encode())
            print(f"ntff profile: {n} file(s) -> {output_dir}", file=sys.stderr)

    mod = types.ModuleType("antenv.axon_hooks")
    mod.get_axon_ntff_profile_hook = lambda: _hook
    mod.set_axon_ntff_profile_hook = lambda h: None
    import antenv
    sys.modules["antenv.axon_hooks"] = mod
    antenv.axon_hooks = mod
    import concourse.bass_utils as bu
    bu.upload_artifacts = lambda tmpdir: tmpdir


def _build_and_run(r_full, wpack, trace):
    sys.path.insert(0, "/opt/trn_rl_repo")
    import concourse.bass as bass
    import concourse.tile as tile
    from concourse import bacc, mybir
    from concourse.bass_utils import run_bass_kernel_spmd

    f32, f16 = mybir.dt.float32, mybir.dt.float16
    AF = mybir.ActivationFunctionType
    ALU = mybir.AluOpType

    # Pin all activations (Sin/Tanh/Relu) to the one table set that holds
    # them all -- otherwise the table-load placement pass alternates sets
    # between Sin and Tanh every tile (~2.7us per switch).
    from concourse.hw_specs import get_activation_tables
    for _name, _funcs in get_activation_tables("gen3").items():
        if _name not in ("silu_and_others",):
            _funcs.discard(AF.Sin)
            _funcs.discard(AF.Tanh)

    NCH = 4                       # interleaved point-chains per iteration
    IT = NCH * NT                 # 2048 points per iteration
    NITER = NCORE // IT           # 64

    # Evacuation engine per (chain, stage): DVE gets 21 of 36, ACT 15
    # (ACT also runs the big Sin and the tanh outputs).
    EVAC_ENG = {}
    for c in range(NCH):
        for L in range(9):
            if c in (0, 1):
                EVAC_ENG[(c, L)] = "v"
            elif c == 2:
                EVAC_ENG[(c, L)] = "s"
            else:
                EVAC_ENG[(c, L)] = "v" if L < 2 else "s"

    nc = bacc.Bacc("TRN2", target_bir_lowering=False, debug=False,
                   num_devices=NCORES)
    r_dram = nc.declare_dram_parameter("r", [91, NCORE], f32, isOutput=False)
    w_dram = nc.declare_dram_parameter("w", [128, W_COLS], f16, isOutput=False)
    oc_dram = nc.declare_dram_parameter("oc", [3, NCORE], f16, isOutput=True)
    os_dram = nc.declare_dram_parameter("os", [1, NCORE], f16, isOutput=True)

    def w_ap(wtile, name):
        off, k, m, base = W_OFF[name]
        return wtile[base:base + k, off:off + m]

    with tile.TileContext(nc) as tc:
        with (
            tc.tile_pool(name="wpool", bufs=1) as wpool,
            tc.tile_pool(name="rpool", bufs=4) as rpool,
            tc.tile_pool(name="epool", bufs=4) as epool,
            tc.tile_pool(name="hpool", bufs=6) as hpool,
            tc.tile_pool(name="cpool", bufs=4) as cpool,
            tc.tile_pool(name="opool", bufs=4) as opool,
            tc.tile_pool(name="p0", bufs=2, space=bass.MemorySpace.PSUM) as pp0,
            tc.tile_pool(name="p1", bufs=2, space=bass.MemorySpace.PSUM) as pp1,
            tc.tile_pool(name="p2", bufs=2, space=bass.MemorySpace.PSUM) as pp2,
            tc.tile_pool(name="p3", bufs=2, space=bass.MemorySpace.PSUM) as pp3,
        ):
            ppools = [pp0, pp1, pp2, pp3]
            wtile = wpool.tile([128, W_COLS], f16)
            nc.sync.dma_start(wtile[:], w_dram[:])

            prev = None   # deferred color head input
            def color_head(prev):
                c65p, Sp, itp = prev
                ot3 = opool.tile([35, IT // 2], f16, tag="ot3")
                t3a = opool.tile([35, IT // 2], f32, tag="t3a")
                # Stack pairs of chains at partition bases 0 and 32 of one
                # PSUM tile so two tanh ops cover all four chains.
                for pr in range(2):
                    hc = bass.ts(pr, IT // 4)   # cols for this pair in t3a
                    po = ppools[pr].tile([35, NT], f32, tag=f"p{pr}")
                    for half in range(2):
                        c = 2 * pr + half
                        cs = bass.ts(c, NT)
                        base = 32 * half
                        nc.tensor.matmul(po[base:base + 3, :],
                                         w_ap(wtile, "w10"), c65p[0:64, cs],
                                         start=True, stop=True,
                                         skip_group_check=True)
                    nc.scalar.activation(t3a[:, hc], po[:], AF.Tanh, scale=0.5)
                nc.gpsimd.tensor_scalar(ot3[:], t3a[:], 0.5, 0.5,
                                        ALU.mult, ALU.add)
                for pr in range(2):
                    for half in range(2):
                        c = 2 * pr + half
                        base = 32 * half
                        col = pr * NT
                        nc.sync.dma_start(
                            oc_dram[:, bass.ts(itp * NCH + c, NT)],
                            ot3[base:base + 3, col:col + NT])
                nc.sync.dma_start(os_dram[:, Sp], c65p[64:65, :])

            for it in range(NITER):
                S = bass.ts(it, IT)
                rt = rpool.tile([91, IT], f32, tag="rt")
                nc.sync.dma_start(rt[:], r_dram[:, S])
                et = epool.tile([91, IT], f16, tag="et")
                nc.scalar.activation(et[:], rt[:], AF.Sin)

                if prev is not None:
                    color_head(prev)

                x_emb = [et[0:63, bass.ts(c, NT)] for c in range(NCH)]
                d_emb = [et[64:91, bass.ts(c, NT)] for c in range(NCH)]

                h = None      # shared [128, IT] tile, one per stage
                c65 = None    # shared [65, IT]

                def stage_mm(c, L):
                    pool = ppools[c]
                    cs = bass.ts(c, NT)
                    if L == 0:
                        p = pool.tile([128, NT], f32, tag=f"p{c}")
                        nc.tensor.matmul(p[:], w_ap(wtile, "w0"), x_emb[c],
                                         start=True, stop=True)
                    elif L in (1, 2, 3, 4, 6, 7):
                        p = pool.tile([128, NT], f32, tag=f"p{c}")
                        nc.tensor.matmul(p[:], w_ap(wtile, f"w{L}"),
                                         h[:, cs], start=True, stop=True)
                    elif L == 5:
                        p = pool.tile([128, NT], f32, tag=f"p{c}")
                        nc.tensor.matmul(p[:], w_ap(wtile, "w5a"), h[:, cs],
                                         start=True, stop=False)
                        nc.tensor.matmul(p[:], w_ap(wtile, "w5b"), x_emb[c],
                                         start=False, stop=True)
                    else:  # L == 8: c_pre [65, NT] (+ sigma row)
                        p = pool.tile([65, NT], f32, tag=f"p{c}")
                        nc.tensor.matmul(p[:], w_ap(wtile, "w8"), h[:, cs],
                                         start=True, stop=False)
                        nc.tensor.matmul(p[:], w_ap(wtile, "w9"), d_emb[c],
                                         start=False, stop=True)
                    return p

                for L in range(9):
                    ps = [stage_mm(c, L) for c in range(NCH)]
                    if L == 8:
                        dst = cpool.tile([65, IT], f16, tag="c65")
                    else:
                        dst = hpool.tile([128, IT], f16, tag="h")
                    for c in range(NCH):
                        cs = bass.ts(c, NT)
                        if EVAC_ENG[(c, L)] == "v":
                            nc.vector.tensor_scalar_max(dst[:, cs], ps[c][:], 0.0)
                        else:
                            nc.scalar.activation(dst[:, cs], ps[c][:], AF.Relu)
                    if L == 8:
                        c65 = dst
                    else:
                        h = dst

                prev = (c65, S, it)

            color_head(prev)

    nc.compile()

    in_maps = []
    for c in range(NCORES):
        rc = np.ascontiguousarray(r_full[:, c * NCORE:(c + 1) * NCORE])
        in_maps.append({"r": rc, "w": wpack})
    kw = {}
    if trace:
        _install_trace_shims()
        trace_dir = os.environ.get("FFNERF_TRACE_DIR", "/tmp/ffnerf_trace")
        os.makedirs(trace_dir, exist_ok=True)
        kw["tmpdir"] = trace_dir
    res = run_bass_kernel_spmd(nc, in_maps, list(range(NCORES)), trace=trace, **kw)
    if trace and res.instructions_and_trace is not None:
        print("perfetto trace:", res.instructions_and_trace[1], file=sys.stderr)
    oc = np.concatenate([res.results[c]["oc"] for c in range(NCORES)], axis=1)
    osig = np.concatenate([res.results[c]["os"] for c in range(NCORES)], axis=1)
    return oc, osig, res


def _run_device_subprocess(r_full, wpack, trace):
    tmpdir = tempfile.mkdtemp(prefix="ffnerf_")
    in_path = os.path.join(tmpdir, "in.npz")
    out_path = os.path.join(tmpdir, "out.npz")
    np.savez(in_path, r=r_full, w=wpack)
    env = dict(os.environ)
    env.pop("JAX_PLATFORMS", None)
    env["FFNERF_TRACE"] = "1" if trace else "0"
    cmd = [sys.executable, os.path.abspath(__file__), "--device-run",
           in_path, out_path]
    # Device sessions occasionally fail transiently (e.g. a prior session
    # still releasing the cores) -- retry a few times.
    import time
    last = None
    for attempt in range(4):
        try:
            subprocess.run(cmd, check=True, env=env)
            break
        except subprocess.CalledProcessError as e:
            last = e
            time.sleep(10 * (attempt + 1))
    else:
        raise last
    d = np.load(out_path)
    return d["oc"], d["os"], d["exec_ns"]


def kernel(pos, dirs, b1_w0, b1_w1, b1_w2, b1_w3, b1_w4,
           b2_w0, b2_w1, b2_w2, b2_w3, b3_w0, b3_w1):
    pos = np.asarray(pos, dtype=np.float32)
    dirs = np.asarray(dirs, dtype=np.float32)
    assert pos.shape == (N, 3) and dirs.shape == (N, 3)
    wpack = _prep_weights(b1_w0, b1_w1, b1_w2, b1_w3, b1_w4,
                          b2_w0, b2_w1, b2_w2, b2_w3, b3_w0, b3_w1)
    r_full = _build_r(pos, dirs)
    trace = os.environ.get("FFNERF_TRACE", "0") == "1"
    oc, osig, exec_ns = _run_device_subprocess(r_full, wpack, trace)
    global LAST_EXEC_NS
    LAST_EXEC_NS = exec_ns
    color = np.ascontiguousarray(oc.T).astype(np.float32)        # [N, 3]
    sigma = osig[0].astype(np.float32)                           # [N]
    return color, sigma


LAST_EXEC_NS = None


if __name__ == "__main__" and len(sys.argv) >= 4 and sys.argv[1] == "--device-run":
    d = np.load(sys.argv[2])
    trace = os.environ.get("FFNERF_TRACE", "0") == "1"
    oc, osig, res = _build_and_run(d["r"], d["w"], trace)
    exec_ns = res.exec_time_ns if res.exec_time_ns is not None else -1
    np.savez(sys.argv[3], oc=oc, os=osig, exec_ns=np.int64(exec_ns))
    print(f"device-run done, exec_time_ns={exec_ns}")


# revision 24
# speedup vs baseline: 1.1720x; 1.0463x over previous
"""Trainium2 Bass kernel for the FF-NeRF MLP (nn_FFNerfModel_3092376453816).

Data-parallel over 8 NeuronCores: points sharded along N, weights replicated.
Feature-major on-chip layout: activations are [d, n_tile] with the feature dim
on SBUF partitions, so every layer is a single TensorE matmul with the weight
as the stationary operand and no transposes anywhere.

Positional encoding: the host pre-computes exactly-wrapped sin arguments
(2^j * x mod 2pi, in float64 -- exact because fp32(2^j*x) is exact), so the
whole embedding is ONE ScalarE Sin op per 2048-point iteration (the HW sin
table only covers [-pi, pi]).  Raw coords ride along as sin(eps*x)/eps with
the 1/eps folded into the first-layer weights.

Per 2048-point iteration, FOUR independent 512-point chains are interleaved
(each with its own 2-slot PSUM pool; 8 banks total) so the serial
matmul->PSUM-evacuation dependency chain of one chain overlaps the others;
PSUM evacuation (relu+fp16 cast) is split between VectorE and ScalarE.  All
matmuls are fp16 (fp32 is 4x slower on the PE); PSUM accumulates fp32.  The
b2_w3 @ b3_w0[:128] linear pair is fused host-side, and sigma rides as
column 65 of that fused matmul, so block2's output layer and block3's input
layer cost one matmul stream.  sigmoid(x) = 0.5 + 0.5*tanh(x/2) keeps the
whole kernel on one activation-table set (sin+tanh+relu in silu_and_others).

Measured: ~1.12 ms NEFF exec on 8 cores, color l2 rel-err 2.2e-4,
sigma 7.9e-4 vs the fp32 reference.
"""

import os
import subprocess
import sys
import tempfile

import numpy as np

N = 1048576
NCORES = 8
NCORE = N // NCORES          # 131072 points per core
NT = 512                     # points per tile (one PSUM bank of fp32)
NTILES = NCORE // NT         # 256
L_POS, L_DIR, H = 10, 4, 128
D_POS = 3 + 2 * 3 * L_POS    # 63
D_DIR = 3 + 2 * 3 * L_DIR    # 27
EPS = 2.0 ** -8              # raw-coord sin trick: sin(eps*x) ~= eps*x
TWO_PI = 2.0 * np.pi

# Rows of the on-chip embedding tile E [90, n]:
#   0..2    eps*pos               (raw coords via sin-linearization)
#   3..32   sin(2^j pos_c)        r = 3 + 3j + c
#   33..62  cos(2^j pos_c)        r = 33 + 3j + c
#   63      zero pad (AP base-partition must be 0/32/64)
#   64..66  eps*dirs
#   67..78  sin(2^j dirs_c)
#   79..90  cos(2^j dirs_c)
# x_emb = rows 0..62 (contiguous), d_emb = rows 64..90 (contiguous).


def _perm(L):
    """my-local-row -> reference-row for one posenc block of size 3+6L."""
    d = 3 + 6 * L
    p = np.zeros(d, dtype=np.int64)
    p[0:3] = [0, 1, 2]
    for j in range(L):
        for c in range(3):
            p[3 + 3 * j + c] = 3 + 6 * j + c          # sin rows
            p[3 + 3 * L + 3 * j + c] = 6 + 6 * j + c  # cos rows
    return p


def _permute_w(w_ref, L):
    """Permute posenc-facing weight rows to my layout; scale raw rows by 1/eps."""
    w = np.asarray(w_ref, dtype=np.float64)[_perm(L)]
    w[0:3] *= 1.0 / EPS
    return w


def _build_r(pos, dirs):
    """R [90, N] float32: pre-wrapped sin arguments for the big Sin op."""
    r = np.empty((91, pos.shape[0]), dtype=np.float32)
    r[63] = 0.0
    pos_t = np.asarray(pos, dtype=np.float64).T     # [3, N]
    dir_t = np.asarray(dirs, dtype=np.float64).T

    def wrap(t):
        return (t - TWO_PI * np.round(t / TWO_PI)).astype(np.float32)

    r[0:3] = (EPS * pos_t).astype(np.float32)
    r[64:67] = (EPS * dir_t).astype(np.float32)
    for j in range(L_POS):
        arg = (2.0 ** j) * pos_t
        r[3 + 3 * j: 6 + 3 * j] = wrap(arg)
        r[33 + 3 * j: 36 + 3 * j] = wrap(arg + np.pi / 2)
    for j in range(L_DIR):
        arg = (2.0 ** j) * dir_t
        r[67 + 3 * j: 70 + 3 * j] = wrap(arg)
        r[79 + 3 * j: 82 + 3 * j] = wrap(arg + np.pi / 2)
    return r


# Column offsets of each stationary weight inside the packed [128, 1285] tile.
W_OFF = {
    "w0": (0, 63, 128, 0),      # (col offset, K, M, base partition)
    "w1": (128, 128, 128, 0),
    "w2": (256, 128, 128, 0),
    "w3": (384, 128, 128, 0),
    "w4": (512, 128, 128, 0),
    "w5a": (640, 128, 128, 0),
    "w5b": (768, 63, 128, 0),
    "w6": (896, 128, 128, 0),
    "w7": (1024, 128, 128, 0),
    "w8": (1152, 128, 65, 0),
    "w9": (1217, 27, 65, 64),   # moving operand d_emb starts at partition 64
    "w10": (1282, 64, 3, 0),
}
W_COLS = 1285


def _pack_weights(ws):
    wpack = np.zeros((128, W_COLS), dtype=np.float16)
    for name, mat in ws.items():
        off, k, m, base = W_OFF[name]
        assert mat.shape == (k, m), (name, mat.shape)
        wpack[base:base + k, off:off + m] = mat.astype(np.float16)
    return wpack


def _prep_weights(b1_w0, b1_w1, b1_w2, b1_w3, b1_w4,
                  b2_w0, b2_w1, b2_w2, b2_w3, b3_w0, b3_w1):
    f64 = lambda a: np.asarray(a, dtype=np.float64)
    # Fuse the two linear layers around `feat`: feat = g2 @ b2_w3[:, :128]
    # (no activation) feeds b3_w0[:128] directly.
    w_fused = f64(b2_w3)[:, :128] @ f64(b3_w0)[:128, :]      # [128, 64]
    w8 = np.concatenate([w_fused, f64(b2_w3)[:, 128:129]], axis=1)  # [128, 65]
    w9 = np.zeros((27, 65), dtype=np.float64)
    w9[:, 0:64] = _permute_w(f64(b3_w0)[128:], L_DIR)        # d_emb -> c
    ws = {
        "w0": _permute_w(b1_w0, L_POS),
        "w1": f64(b1_w1), "w2": f64(b1_w2), "w3": f64(b1_w3), "w4": f64(b1_w4),
        "w5a": f64(b2_w0)[:128], "w5b": _permute_w(f64(b2_w0)[128:], L_POS),
        "w6": f64(b2_w1), "w7": f64(b2_w2),
        "w8": w8, "w9": w9, "w10": f64(b3_w1),
    }
    return _pack_weights(ws)


# ---------------------------------------------------------------------------
# Device side (runs in a subprocess so the host process's jax state/platform
# selection can't interfere with the axon PJRT backend).
# ---------------------------------------------------------------------------

def _install_trace_shims():
    """The image's antenv lacks axon_hooks; replicate trn_boot's ctypes hook
    so run_bass_kernel_spmd(trace=True) can capture NTFF profiles."""
    import contextlib
    import ctypes
    import types

    lib = ctypes.CDLL("/opt/axon/libaxon_pjrt.so")
    if not hasattr(lib, "axon_start_nrt_profile"):
        return
    lib.axon_start_nrt_profile.argtypes = [ctypes.POINTER(ctypes.c_int64),
                                           ctypes.c_size_t]
    lib.axon_start_nrt_profile.restype = ctypes.c_int64
    lib.axon_stop_nrt_profile.argtypes = [ctypes.c_char_p]
    lib.axon_stop_nrt_profile.restype = ctypes.c_int64

    @contextlib.contextmanager
    def _hook(output_dir, device_ids):
        import jax
        jax.devices()
        if device_ids:
            ids = (ctypes.c_int64 * len(device_ids))(*device_ids)
            rc = lib.axon_start_nrt_profile(ids, len(device_ids))
        else:
            rc = lib.axon_start_nrt_profile(None, 0)
        if rc != 0:
            raise RuntimeError(f"axon_start_nrt_profile rc={rc}")
        try:
            yield
        finally:
            n = lib.axon_stop_nrt_profile(str(output_dir).encode())
            print(f"ntff profile: {n} file(s) -> {output_dir}", file=sys.stderr)

    mod = types.ModuleType("antenv.axon_hooks")
    mod.get_axon_ntff_profile_hook = lambda: _hook
    mod.set_axon_ntff_profile_hook = lambda h: None
    import antenv
    sys.modules["antenv.axon_hooks"] = mod
    antenv.axon_hooks = mod
    import concourse.bass_utils as bu
    bu.upload_artifacts = lambda tmpdir: tmpdir


def _build_and_run(r_full, wpack, trace):
    sys.path.insert(0, "/opt/trn_rl_repo")
    import concourse.bass as bass
    import concourse.tile as tile
    from concourse import bacc, mybir
    from concourse.bass_utils import run_bass_kernel_spmd

    f32, f16 = mybir.dt.float32, mybir.dt.float16
    AF = mybir.ActivationFunctionType
    ALU = mybir.AluOpType

    # Pin all activations (Sin/Tanh/Relu) to the one table set that holds
    # them all -- otherwise the table-load placement pass alternates sets
    # between Sin and Tanh every tile (~2.7us per switch).
    from concourse.hw_specs import get_activation_tables
    for _name, _funcs in get_activation_tables("gen3").items():
        if _name not in ("silu_and_others",):
            _funcs.discard(AF.Sin)
            _funcs.discard(AF.Tanh)

    NCH = 4                       # interleaved point-chains per iteration
    IT = NCH * NT                 # 2048 points per iteration
    NITER = NCORE // IT           # 64

    # Evacuation engine per (chain, stage): DVE gets 21 of 36, ACT 15
    # (ACT also runs the big Sin and the tanh outputs).
    EVAC_ENG = {}
    for c in range(NCH):
        for L in range(9):
            if c in (0, 1):
                EVAC_ENG[(c, L)] = "v"
            elif c == 2:
                EVAC_ENG[(c, L)] = "v" if L < 2 else "s"
            else:
                EVAC_ENG[(c, L)] = "s"

    nc = bacc.Bacc("TRN2", target_bir_lowering=False, debug=False,
                   num_devices=NCORES)
    r_dram = nc.declare_dram_parameter("r", [91, NCORE], f32, isOutput=False)
    w_dram = nc.declare_dram_parameter("w", [128, W_COLS], f16, isOutput=False)
    oc_dram = nc.declare_dram_parameter("oc", [3, NCORE], f16, isOutput=True)
    os_dram = nc.declare_dram_parameter("os", [1, NCORE], f16, isOutput=True)

    def w_ap(wtile, name):
        off, k, m, base = W_OFF[name]
        return wtile[base:base + k, off:off + m]

    with tile.TileContext(nc) as tc:
        with (
            tc.tile_pool(name="wpool", bufs=1) as wpool,
            tc.tile_pool(name="rpool", bufs=4) as rpool,
            tc.tile_pool(name="epool", bufs=4) as epool,
            tc.tile_pool(name="hpool", bufs=6) as hpool,
            tc.tile_pool(name="cpool", bufs=4) as cpool,
            tc.tile_pool(name="opool", bufs=4) as opool,
            tc.tile_pool(name="p0", bufs=2, space=bass.MemorySpace.PSUM) as pp0,
            tc.tile_pool(name="p1", bufs=2, space=bass.MemorySpace.PSUM) as pp1,
            tc.tile_pool(name="p2", bufs=2, space=bass.MemorySpace.PSUM) as pp2,
            tc.tile_pool(name="p3", bufs=2, space=bass.MemorySpace.PSUM) as pp3,
        ):
            ppools = [pp0, pp1, pp2, pp3]
            wtile = wpool.tile([128, W_COLS], f16)
            nc.sync.dma_start(wtile[:], w_dram[:])

            prev = None   # deferred color head input
            def color_head(prev):
                c65p, Sp, itp = prev
                ot3 = opool.tile([35, IT // 2], f16, tag="ot3")
                t3a = opool.tile([35, IT // 2], f32, tag="t3a")
                # Stack pairs of chains at partition bases 0 and 32 of one
                # PSUM tile so two tanh ops cover all four chains.
                for pr in range(2):
                    hc = bass.ts(pr, IT // 4)   # cols for this pair in t3a
                    po = ppools[pr].tile([35, NT], f32, tag=f"p{pr}")
                    for half in range(2):
                        c = 2 * pr + half
                        cs = bass.ts(c, NT)
                        base = 32 * half
                        nc.tensor.matmul(po[base:base + 3, :],
                                         w_ap(wtile, "w10"), c65p[0:64, cs],
                                         start=True, stop=True,
                                         skip_group_check=True)
                    nc.scalar.activation(t3a[:, hc], po[:], AF.Tanh, scale=0.5)
                nc.gpsimd.tensor_scalar(ot3[:], t3a[:], 0.5, 0.5,
                                        ALU.mult, ALU.add)
                for pr in range(2):
                    for half in range(2):
                        c = 2 * pr + half
                        base = 32 * half
                        col = pr * NT
                        nc.sync.dma_start(
                            oc_dram[:, bass.ts(itp * NCH + c, NT)],
                            ot3[base:base + 3, col:col + NT])
                nc.sync.dma_start(os_dram[:, Sp], c65p[64:65, :])

            for it in range(NITER):
                S = bass.ts(it, IT)
                rt = rpool.tile([91, IT], f32, tag="rt")
                nc.sync.dma_start(rt[:], r_dram[:, S])
                et = epool.tile([91, IT], f16, tag="et")
                nc.scalar.activation(et[:], rt[:], AF.Sin)

                if prev is not None:
                    color_head(prev)

                x_emb = [et[0:63, bass.ts(c, NT)] for c in range(NCH)]
                d_emb = [et[64:91, bass.ts(c, NT)] for c in range(NCH)]

                h = None      # shared [128, IT] tile, one per stage
                c65 = None    # shared [65, IT]

                def stage_mm(c, L):
                    pool = ppools[c]
                    cs = bass.ts(c, NT)
                    if L == 0:
                        p = pool.tile([128, NT], f32, tag=f"p{c}")
                        nc.tensor.matmul(p[:], w_ap(wtile, "w0"), x_emb[c],
                                         start=True, stop=True)
                    elif L in (1, 2, 3, 4, 6, 7):
                        p = pool.tile([128, NT], f32, tag=f"p{c}")
                        nc.tensor.matmul(p[:], w_ap(wtile, f"w{L}"),
                                         h[:, cs], start=True, stop=True)
                    elif L == 5:
                        p = pool.tile([128, NT], f32, tag=f"p{c}")
                        nc.tensor.matmul(p[:], w_ap(wtile, "w5a"), h[:, cs],
                                         start=True, stop=False)
                        nc.tensor.matmul(p[:], w_ap(wtile, "w5b"), x_emb[c],
                                         start=False, stop=True)
                    else:  # L == 8: c_pre [65, NT] (+ sigma row)
                        p = pool.tile([65, NT], f32, tag=f"p{c}")
                        nc.tensor.matmul(p[:], w_ap(wtile, "w8"), h[:, cs],
                                         start=True, stop=False)
                        nc.tensor.matmul(p[:], w_ap(wtile, "w9"), d_emb[c],
                                         start=False, stop=True)
                    return p

                for L in range(9):
                    ps = [stage_mm(c, L) for c in range(NCH)]
                    if L == 8:
                        dst = cpool.tile([65, IT], f16, tag="c65")
                    else:
                        dst = hpool.tile([128, IT], f16, tag="h")
                    for c in range(NCH):
                        cs = bass.ts(c, NT)
                        if EVAC_ENG[(c, L)] == "v":
                            nc.vector.tensor_scalar_max(dst[:, cs], ps[c][:], 0.0)
                        else:
                            nc.scalar.activation(dst[:, cs], ps[c][:], AF.Relu)
                    if L == 8:
                        c65 = dst
                    else:
                        h = dst

                prev = (c65, S, it)

            color_head(prev)

    nc.compile()

    in_maps = []
    for c in range(NCORES):
        rc = np.ascontiguousarray(r_full[:, c * NCORE:(c + 1) * NCORE])
        in_maps.append({"r": rc, "w": wpack})
    kw = {}
    if trace:
        _install_trace_shims()
        trace_dir = os.environ.get("FFNERF_TRACE_DIR", "/tmp/ffnerf_trace")
        os.makedirs(trace_dir, exist_ok=True)
        kw["tmpdir"] = trace_dir
    res = run_bass_kernel_spmd(nc, in_maps, list(range(NCORES)), trace=trace, **kw)
    if trace and res.instructions_and_trace is not None:
        print("perfetto trace:", res.instructions_and_trace[1], file=sys.stderr)
    oc = np.concatenate([res.results[c]["oc"] for c in range(NCORES)], axis=1)
    osig = np.concatenate([res.results[c]["os"] for c in range(NCORES)], axis=1)
    return oc, osig, res


def _run_device_subprocess(r_full, wpack, trace):
    tmpdir = tempfile.mkdtemp(prefix="ffnerf_")
    in_path = os.path.join(tmpdir, "in.npz")
    out_path = os.path.join(tmpdir, "out.npz")
    np.savez(in_path, r=r_full, w=wpack)
    env = dict(os.environ)
    env.pop("JAX_PLATFORMS", None)
    env["FFNERF_TRACE"] = "1" if trace else "0"
    cmd = [sys.executable, os.path.abspath(__file__), "--device-run",
           in_path, out_path]
    # Device sessions occasionally fail transiently (e.g. a prior session
    # still releasing the cores) -- retry a few times.
    import time
    last = None
    for attempt in range(4):
        try:
            subprocess.run(cmd, check=True, env=env)
            break
        except subprocess.CalledProcessError as e:
            last = e
            time.sleep(10 * (attempt + 1))
    else:
        raise last
    d = np.load(out_path)
    return d["oc"], d["os"], d["exec_ns"]


def kernel(pos, dirs, b1_w0, b1_w1, b1_w2, b1_w3, b1_w4,
           b2_w0, b2_w1, b2_w2, b2_w3, b3_w0, b3_w1):
    pos = np.asarray(pos, dtype=np.float32)
    dirs = np.asarray(dirs, dtype=np.float32)
    assert pos.shape == (N, 3) and dirs.shape == (N, 3)
    wpack = _prep_weights(b1_w0, b1_w1, b1_w2, b1_w3, b1_w4,
                          b2_w0, b2_w1, b2_w2, b2_w3, b3_w0, b3_w1)
    r_full = _build_r(pos, dirs)
    trace = os.environ.get("FFNERF_TRACE", "0") == "1"
    oc, osig, exec_ns = _run_device_subprocess(r_full, wpack, trace)
    global LAST_EXEC_NS
    LAST_EXEC_NS = exec_ns
    color = np.ascontiguousarray(oc.T).astype(np.float32)        # [N, 3]
    sigma = osig[0].astype(np.float32)                           # [N]
    return color, sigma


LAST_EXEC_NS = None


if __name__ == "__main__" and len(sys.argv) >= 4 and sys.argv[1] == "--device-run":
    d = np.load(sys.argv[2])
    trace = os.environ.get("FFNERF_TRACE", "0") == "1"
    oc, osig, res = _build_and_run(d["r"], d["w"], trace)
    exec_ns = res.exec_time_ns if res.exec_time_ns is not None else -1
    np.savez(sys.argv[3], oc=oc, os=osig, exec_ns=np.int64(exec_ns))
    print(f"device-run done, exec_time_ns={exec_ns}")
